# revision 1
# baseline (speedup 1.0000x reference)
"""Trainium2 Bass kernel for Convpass-swin hypernet fused adapter.

Reference computation (per batch sample):
  h      = relu(x @ Wm1 + bm1)                    # [B,H,W,64]
  prompt = mean_hw(h) @ Wm2 + bm2                 # [B,64]  (mean commutes with matmul)
  wflat  = (emb + prompt) @ Wh + bh               # [B,96*96*9]
  xd     = quickgelu(x @ Wd + bd)                 # [B,H,W,96]
  y      = quickgelu(conv3x3(xd, wflat))          # per-sample dynamic grouped conv
  out    = y @ Wu + bu                            # [B,H,W,384]

Sharding: data-parallel over batch B=64 across 8 cores (8 samples/core),
weights replicated. Heavy matmuls run in fp32r (fp32 with 11-bit
mantissa, 1 cycle/row on the PE at N>=256) accumulating in fp32 PSUM;
the 21 MB hypernet matrix Wh is streamed in bf16 to halve its DMA cost.
"""
import numpy as np
import ml_dtypes

import concourse.bass as bass
import concourse.tile as tile
import concourse.mybir as mybir
from concourse import bacc
from concourse.bass_utils import run_bass_kernel_spmd
from concourse.masks import make_identity

F32 = mybir.dt.float32
F32R = mybir.dt.float32r
BF = mybir.dt.bfloat16
AF = mybir.ActivationFunctionType
AX = mybir.AxisListType

# problem constants
B, H, W, C = 64, 28, 28, 384
DIM, E, KK = 96, 64, 3
NCORES = 8
BL = B // NCORES          # samples per core
P = H * W                 # 784 positions per sample
HP = H + 2                # padded spatial
NPOS = BL * P             # 6272 positions per core
WH_COLS = DIM * DIM * 9   # 82944
GRP = 16                  # (o,tap) pairs per psum tile / ACT copy
WH_BF16 = True            # stream the hypernet matrix in bf16 (halves DMA bytes)
WH_DMA = DIM * GRP * (3 if WH_BF16 else 2)  # columns per Wh DMA


def _round_fp32r(a):
    """Round-to-nearest-even fp32 -> fp32r (11-bit mantissa kept)."""
    b = np.ascontiguousarray(a, np.float32).view(np.uint32).astype(np.uint64)
    bb = b + np.uint64(0x7FF) + ((b >> np.uint64(12)) & np.uint64(1))
    return (bb & np.uint64(0xFFFFF000)).astype(np.uint32).view(np.float32)


def build_nc(stop_after=None):
    nc = bacc.Bacc("TRN2", target_bir_lowering=False, debug=False)

    x_d = nc.dram_tensor("x", [NPOS, C], F32, kind="ExternalInput").ap()
    wm1_d = nc.dram_tensor("wm1", [C, E], F32R, kind="ExternalInput").ap()
    wd_d = nc.dram_tensor("wd", [C, DIM], F32R, kind="ExternalInput").ap()
    wm2_d = nc.dram_tensor("wm2", [E, E], F32R, kind="ExternalInput").ap()
    wh_d = nc.dram_tensor("wh", [E + 1, WH_COLS], BF if WH_BF16 else F32R,
                          kind="ExternalInput").ap()
    wu_d = nc.dram_tensor("wu", [DIM + 1, C], F32R, kind="ExternalInput").ap()
    bm1_d = nc.dram_tensor("bm1", [E], F32, kind="ExternalInput").ap()
    bd_d = nc.dram_tensor("bd", [DIM], F32, kind="ExternalInput").ap()
    bpe_d = nc.dram_tensor("bpe", [E], F32, kind="ExternalInput").ap()
    out_d = nc.dram_tensor("out", [NPOS, C], F32, kind="ExternalOutput").ap()

    with tile.TileContext(nc) as tc:
        with (
            tc.tile_pool(name="const", bufs=1) as cp,
            tc.tile_pool(name="persist", bufs=1) as pp,
            tc.tile_pool(name="xin", bufs=3) as xin_p,
            tc.tile_pool(name="wh", bufs=4) as wh_p,
        ):
            # ---- constants ----
            wm1_sb = cp.tile([128, C // 128, E], F32R)
            nc.sync.dma_start(wm1_sb[:], wm1_d.rearrange("(c3 p) e -> p c3 e", p=128))
            wd_sb = cp.tile([128, C // 128, DIM], F32R)
            nc.sync.dma_start(wd_sb[:], wd_d.rearrange("(c3 p) e -> p c3 e", p=128))
            wm2_sb = cp.tile([E, E], F32R)
            nc.sync.dma_start(wm2_sb[:], wm2_d[:])
            wu_sb = cp.tile([DIM + 1, C], F32R)
            nc.sync.dma_start(wu_sb[:], wu_d[:])
            bm1_sb = cp.tile([E, 1], F32)
            nc.sync.dma_start(bm1_sb[:], bm1_d[:])
            bd_sb = cp.tile([DIM, 1], F32)
            nc.sync.dma_start(bd_sb[:], bd_d[:])
            bpe_sb = cp.tile([E, 1], F32)
            nc.sync.dma_start(bpe_sb[:], bpe_d[:])
            ident = cp.tile([128, 128], F32)
            make_identity(nc, ident[:])

            # ---- persistent state ----
            xd_pad = pp.tile([DIM, BL, HP, HP], F32R)     # padded gelu(x@Wd+bd)
            w_all = pp.tile([DIM, BL, DIM, 9], F32R)      # per-sample conv weights [i,b,o,t]
            hsum = pp.tile([E, BL, 2], F32)               # per-(sample,half) relu sums
            hbar = pp.tile([E, BL], F32)
            hbar_r = pp.tile([E, BL], F32R)
            pvec = pp.tile([E + 1, BL], BF if WH_BF16 else F32R)  # (emb+prompt, 1)^T


            # zero xd_pad once (borders must be 0); ACT writes rounded f32r zeros
            nc.scalar.activation(
                xd_pad[:].rearrange("p b r c -> p (b r c)"),
                ident[:DIM, 0:1].to_broadcast((DIM, BL * HP * HP)),
                AF.Copy, scale=0.0)

            # ---- phase 1a: per-sample transpose + meta-h sums ----
            xT_ctx = tc.tile_pool(name="xT", bufs=1)
            xT_p = xT_ctx.__enter__()
            ps1 = tc.tile_pool(name="ps1", bufs=2, space="PSUM")
            ps1_pool = ps1.__enter__()
            xTs = []
            for b in range(BL):
                xTb = xT_p.tile([128, C // 128, P], F32R, name=f"xT{b}")
                xTs.append(xTb)
                xin = xin_p.tile([112, 7, C], F32, tag="xin")
                xsrc = x_d[b * P:(b + 1) * P, :].rearrange("(ch p) c -> p ch c", p=112)
                for q in range(3):
                    nc.sync.dma_start(xin[:, :, q * 128:(q + 1) * 128],
                                      xsrc[:, :, q * 128:(q + 1) * 128])
                for ch in range(7):  # 112-position chunks
                    pt = ps1_pool.tile([128, C // 128, 112], F32, name="pt", tag="pt", bufs=3)
                    for c in range(C // 128):
                        nc.tensor.transpose(pt[:, c, :], xin[:, ch, c * 128:(c + 1) * 128],
                                            ident[:112, :112])
                    nc.vector.tensor_copy(xTb[:, :, ch * 112:(ch + 1) * 112], pt[:])

            # meta-net h sums (kept separate so transposes + x DMA pipeline freely)
            for b in range(BL):
                for h2 in range(2):  # 392-position halves
                    ph = ps1_pool.tile([E, 392], F32, name="ph", tag="ph")
                    for c in range(C // 128):
                        nc.tensor.matmul(ph[:], wm1_sb[:, c, :], xTs[b][:, c, h2 * 392:(h2 + 1) * 392],
                                         start=(c == 0), stop=(c == 2))
                    h_scr = xin_p.tile([E, 392], F32, tag="hscr")
                    nc.scalar.activation(h_scr[:], ph[:], AF.Relu, bias=bm1_sb[:],
                                         accum_out=hsum[:, b, h2:h2 + 1])

            # ---- phase 2: prompt -> pvec ----
            nc.vector.reduce_sum(hbar[:], hsum[:], axis=AX.X)
            nc.scalar.activation(hbar_r[:], hbar[:], AF.Copy, scale=1.0 / P)
            ppm = ps1_pool.tile([E, BL], F32, name="ppm", tag="ppm", bufs=1)
            nc.tensor.matmul(ppm[:], wm2_sb[:], hbar_r[:], start=True, stop=True)
            nc.scalar.activation(pvec[:E, :], ppm[:], AF.Identity, bias=bpe_sb[:])
            nc.scalar.activation(pvec[E:E + 1, :], ident[0:1, 0:1].to_broadcast((1, BL)), AF.Copy)

            if stop_after == "1a":
                ps1.__exit__(None, None, None); xT_ctx.__exit__(None, None, None)
                nc.compile(); return nc
            # ---- phase 1b: xd = quickgelu(x@Wd+bd), overlaps Wh streaming ----
            for b in range(BL):
                for h2 in range(2):
                    px = ps1_pool.tile([DIM, 392], F32, name="px", tag="px")
                    for c in range(C // 128):
                        nc.tensor.matmul(px[:], wd_sb[:, c, :], xTs[b][:, c, h2 * 392:(h2 + 1) * 392],
                                         start=(c == 0), stop=(c == 2))
                    nc.scalar.activation(
                        xd_pad[:, b, 1 + h2 * 14: 15 + h2 * 14, 1:29],
                        px[:].rearrange("p (r c) -> p r c", r=14),
                        AF.Gelu_apprx_sigmoid, bias=bd_sb[:])
            ps1.__exit__(None, None, None)
            xT_ctx.__exit__(None, None, None)

            # ---- phase 3: wflat = pvec_aug @ Wh_aug, written straight into
            # w_all via [i,b]-oriented matmuls (one per (o,tap) pair) ----
            ps2 = tc.tile_pool(name="ps2", bufs=6, space="PSUM")
            ps2_pool = ps2.__enter__()
            w_gb = w_all[:].rearrange("i b o t -> i (o t) b")  # [96, 864, 8] view
            for jd in range(WH_COLS // WH_DMA):  # 27 DMA chunks
                whc = wh_p.tile([E + 1, WH_DMA], BF if WH_BF16 else F32R, tag="whc")
                nc.sync.dma_start(whc[:], wh_d[:, jd * WH_DMA:(jd + 1) * WH_DMA])
                for grp in range(WH_DMA // (DIM * GRP)):  # 4 copy-groups
                    pwg = ps2_pool.tile([DIM, GRP * BL], F32, name="pwg", tag="pwg")
                    for gg in range(GRP):
                        col = (grp * GRP + gg) * DIM
                        nc.tensor.matmul(pwg[:, gg * BL:(gg + 1) * BL],
                                         whc[:, col:col + DIM], pvec[:],
                                         start=True, stop=True)
                    g0 = jd * (WH_DMA // DIM) + grp * GRP
                    if (jd + grp) % 2 == 0:
                        nc.scalar.activation(
                            w_gb[:, g0:g0 + GRP, :],
                            pwg[:].rearrange("i (g b) -> i g b", g=GRP),
                            AF.Copy)
                    else:
                        nc.vector.tensor_copy(
                            w_gb[:, g0:g0 + GRP, :],
                            pwg[:].rearrange("i (g b) -> i g b", g=GRP))

            ps2.__exit__(None, None, None)
            if stop_after == "3":
                nc.compile(); return nc

            # ---- phase 4+5: per-sample conv, gelu, unadapter matmul ----
            yp_ctx = tc.tile_pool(name="yp", bufs=1)
            yp = yp_ctx.__enter__()
            outp_ctx = tc.tile_pool(name="outp", bufs=4)
            out_p = outp_ctx.__enter__()
            y_bs = [yp.tile([DIM + 1, P], F32R, name=f"y_b{b}") for b in range(BL)]
            ps3 = tc.tile_pool(name="ps3", bufs=4, space="PSUM")
            ps3_pool = ps3.__enter__()
            for b in range(BL):
                yb = y_bs[b]
                for h2 in range(2):
                    py = ps3_pool.tile([DIM, 392], F32, name="py", tag="py")
                    for t in range(9):
                        dy, dx = t // 3, t % 3
                        nc.tensor.matmul(
                            py[:],
                            w_all[:, b, :, t],
                            xd_pad[:, b, h2 * 14 + dy: h2 * 14 + dy + 14, dx:dx + 28],
                            start=(t == 0), stop=(t == 8))
                    nc.scalar.activation(yb[:DIM, h2 * 392:(h2 + 1) * 392], py[:],
                                         AF.Gelu_apprx_sigmoid)
                nc.scalar.activation(yb[DIM:DIM + 1, :], ident[0:1, 0:1].to_broadcast((1, P)),
                                     AF.Copy)
                ob = out_p.tile([112, 7, C], F32, tag="ob")
                for m in range(7):  # 112-position chunks
                    po = ps3_pool.tile([112, C], F32, name="po", tag="po")
                    nc.tensor.matmul(po[:], yb[:, m * 112:(m + 1) * 112], wu_sb[:],
                                     start=True, stop=True)
                    nc.vector.tensor_copy(ob[:, m, :], po[:])
                odst = out_d[b * P:(b + 1) * P, :].rearrange("(m p) c -> p m c", p=112)
                for q, r in ((0, 2), (2, 4), (4, 7)):
                    nc.sync.dma_start(odst[:, q:r, :], ob[:, q:r, :])
            ps3.__exit__(None, None, None)
            outp_ctx.__exit__(None, None, None)
            yp_ctx.__exit__(None, None, None)

    nc.compile()
    return nc


_NC_CACHE = None


def _get_nc():
    global _NC_CACHE
    if _NC_CACHE is None:
        _NC_CACHE = build_nc()
    return _NC_CACHE


def _prep_inputs(x, Wd, bd, Wm1, bm1, Wm2, bm2, Wh, bh, emb, Wu, bu):
    """Host-side prep: permute/augment weights, round to fp32r, shard x."""
    whp = np.asarray(Wh, np.float32).reshape(E, DIM, DIM, 9)      # (e, o, i, t)
    whp = whp.transpose(0, 1, 3, 2).reshape(E, WH_COLS)           # (e, (o, t, i))
    bhp = np.asarray(bh, np.float32).reshape(DIM, DIM, 9)
    bhp = bhp.transpose(0, 2, 1).reshape(WH_COLS)
    wh_aug = np.concatenate([whp, bhp[None]], 0)                  # [65, WH_COLS]
    wu_aug = np.concatenate([np.asarray(Wu, np.float32),
                             np.asarray(bu, np.float32)[None]], 0)  # [97, C]
    shared = {
        "wm1": _round_fp32r(Wm1),
        "wd": _round_fp32r(Wd),
        "wm2": _round_fp32r(Wm2),
        "wh": (wh_aug.astype(ml_dtypes.bfloat16) if WH_BF16 else _round_fp32r(wh_aug)),
        "wu": _round_fp32r(wu_aug),
        "bm1": np.ascontiguousarray(bm1, np.float32),
        "bd": np.ascontiguousarray(bd, np.float32),
        "bpe": np.ascontiguousarray(np.asarray(bm2, np.float32)
                                    + np.asarray(emb, np.float32)),
    }
    xs = np.ascontiguousarray(np.asarray(x, np.float32).reshape(B, P, C))
    in_maps = []
    for k in range(NCORES):
        m = dict(shared)
        m["x"] = np.ascontiguousarray(xs[k * BL:(k + 1) * BL].reshape(NPOS, C))
        in_maps.append(m)
    return in_maps


def _run(inputs, **spmd_kwargs):
    nc = _get_nc()
    in_maps = _prep_inputs(**inputs)
    res = run_bass_kernel_spmd(nc, in_maps, core_ids=list(range(NCORES)), **spmd_kwargs)
    out = np.concatenate([r["out"] for r in res.results], 0)
    return out.reshape(B, H, W, C), res


def kernel(**inputs) -> np.ndarray:
    out, _ = _run(inputs)
    return out



# revision 6
# speedup vs baseline: 1.2431x; 1.2431x over previous
"""Trainium2 Bass kernel for Convpass-swin hypernet fused adapter.

Reference computation (per batch sample):
  h      = relu(x @ Wm1 + bm1)                    # [B,H,W,64]
  prompt = mean_hw(h) @ Wm2 + bm2                 # [B,64]  (mean commutes with matmul)
  wflat  = (emb + prompt) @ Wh + bh               # [B,96*96*9]
  xd     = quickgelu(x @ Wd + bd)                 # [B,H,W,96]
  y      = quickgelu(conv3x3(xd, wflat))          # per-sample dynamic grouped conv
  out    = y @ Wu + bu                            # [B,H,W,384]

Sharding: data-parallel over batch B=64 across 8 cores (8 samples/core),
weights replicated.

Key layout choices (all host-side prep is x-independent except pure
dtype/layout casts of x itself):
  * x is uploaded pre-transposed as [C_part=128, 3, BL, P] fp16, so no
    on-chip transposes are needed and the meta/adapter matmuls read it
    directly as the moving tensor.
  * The hypernet constant term w0 = (emb+bm2)@Wh + bh is folded into an
    extra row of the (o,t,i)-permuted Wh, streamed in fp16; the device
    computes wflat = prompt_raw @ Wh + w0 via an augmented [65,BL] pvec.
  * The final projection runs with C on partitions (stationary Wu^T
    chunks), so bu is a per-partition ACT bias and the output is written
    back as [C_part, 3, BL, P] fp16 (host transposes back).
"""
import numpy as np

import concourse.bass as bass
import concourse.tile as tile
import concourse.mybir as mybir
from concourse import bacc
from concourse.bass_utils import run_bass_kernel_spmd

F32 = mybir.dt.float32
F16 = mybir.dt.float16
AF = mybir.ActivationFunctionType
AX = mybir.AxisListType

# problem constants
B, H, W, C = 64, 28, 28, 384
DIM, E, KK = 96, 64, 3
NCORES = 8
BL = B // NCORES          # samples per core
P = H * W                 # 784 positions per sample
HP = H + 2                # padded spatial
WH_COLS = DIM * DIM * 9   # 82944
NCH = 27                  # Wh stream chunks
CHW = WH_COLS // NCH      # 3072 columns per chunk
GRP = CHW // DIM          # 32 (o,t) groups per chunk
HF = P // 2               # 392 positions per half


def build_nc():
    nc = bacc.Bacc("TRN2", target_bir_lowering=False, debug=False)

    xt_d = nc.dram_tensor("xt", [128, 3, BL, P], F16, kind="ExternalInput").ap()
    w16_d = nc.dram_tensor("w16", [128, 552], F16, kind="ExternalInput").ap()
    wu_d = nc.dram_tensor("wu", [DIM, C], F16, kind="ExternalInput").ap()
    bias_d = nc.dram_tensor("bias", [128, 5], F32, kind="ExternalInput").ap()
    wh_d = nc.dram_tensor("wh", [E + 1, WH_COLS], F16, kind="ExternalInput").ap()
    yo_d = nc.dram_tensor("yo", [128, 3, BL, P], F16, kind="ExternalOutput").ap()

    with tile.TileContext(nc) as tc:
        with (
            tc.tile_pool(name="const", bufs=1) as cp,
            tc.tile_pool(name="persist", bufs=1) as pp,
            tc.tile_pool(name="scr", bufs=3) as sp,
            tc.tile_pool(name="wh", bufs=8) as wh_p,
            tc.tile_pool(name="ob", bufs=3) as ob_p,
        ):
            # ---- constants (one small DMA each) ----
            w16_sb = cp.tile([128, 552], F16)
            nc.sync.dma_start(w16_sb[:], w16_d[:])
            wu_sb = cp.tile([DIM, C], F16)
            nc.sync.dma_start(wu_sb[:], wu_d[:])
            bias_sb = cp.tile([128, 5], F32)
            nc.sync.dma_start(bias_sb[:], bias_d[:])

            # ---- x stream, one DMA per sample ----
            xt_sb = pp.tile([128, 3, BL, P], F16)
            for b in range(BL):
                nc.sync.dma_start(xt_sb[:, :, b, :], xt_d[:, :, b, :])

            # ---- persistent state ----
            xd_pad = pp.tile([DIM, BL, HP, HP], F16)      # padded gelu(x@Wd+bd)
            w_all = pp.tile([DIM, BL, DIM, 9], F16)       # per-sample conv weights [i,b,o,t]
            y_all = pp.tile([DIM, BL, P], F16)            # gelu(conv) activations
            hsum = pp.tile([E, BL, 2], F32)               # per-(sample,half) relu sums
            hbar = pp.tile([E, BL], F32)
            hbar16 = pp.tile([E, BL], F16)
            pvec = pp.tile([E + 1, BL], F16)              # (prompt_raw, 1)^T

            # zero only the conv halo ring; phase 1b writes the interior
            nc.vector.memset(xd_pad[:, :, 0:1, :], 0.0)
            nc.vector.memset(xd_pad[:, :, HP - 1:HP, :], 0.0)
            nc.vector.memset(xd_pad[:, :, 1:HP - 1, 0:1], 0.0)
            nc.vector.memset(xd_pad[:, :, 1:HP - 1, HP - 1:HP], 0.0)
            nc.vector.memset(pvec[E:E + 1, :], 1.0)

            # ---- phase 1a: meta h = relu(x@Wm1+bm1), spatial sums ----
            ps1 = tc.tile_pool(name="ps1", bufs=4, space="PSUM")
            p1 = ps1.__enter__()
            for b in range(BL):
                for h2 in range(2):
                    ph = p1.tile([E, HF], F32, name="ph", tag="ph", bufs=3)
                    for c in range(3):
                        nc.tensor.matmul(ph[:], w16_sb[:, c * 64:(c + 1) * 64],
                                         xt_sb[:, c, b, h2 * HF:(h2 + 1) * HF],
                                         start=(c == 0), stop=(c == 2))
                    h_scr = sp.tile([E, HF], F32, tag="hscr")
                    nc.scalar.activation(h_scr[:], ph[:], AF.Relu,
                                         bias=bias_sb[0:E, 0:1],
                                         accum_out=hsum[:, b, h2:h2 + 1])

            # ---- phase 2: prompt_raw -> pvec ----
            nc.vector.reduce_sum(hbar[:], hsum[:], axis=AX.X)
            nc.scalar.activation(hbar16[:], hbar[:], AF.Copy, scale=1.0 / P)
            ppm = p1.tile([E, BL], F32, name="ppm", tag="ppm", bufs=1)
            nc.tensor.matmul(ppm[:], w16_sb[0:E, 480:544], hbar16[:],
                             start=True, stop=True)
            nc.scalar.activation(pvec[0:E, :], ppm[:], AF.Copy)

            # ---- phase 1b: xd = quickgelu(x@Wd+bd) into padded layout ----
            for b in range(BL):
                for h2 in range(2):
                    px = p1.tile([DIM, HF], F32, name="px", tag="px", bufs=3)
                    for c in range(3):
                        nc.tensor.matmul(px[:], w16_sb[:, 192 + c * 96:192 + (c + 1) * 96],
                                         xt_sb[:, c, b, h2 * HF:(h2 + 1) * HF],
                                         start=(c == 0), stop=(c == 2))
                    nc.scalar.activation(
                        xd_pad[:, b, 1 + h2 * 14:15 + h2 * 14, 1:29],
                        px[:].rearrange("p (r c) -> p r c", r=14),
                        AF.Gelu_apprx_sigmoid, bias=bias_sb[0:DIM, 1:2])
            ps1.__exit__(None, None, None)

            # ---- phase 3: wflat = pvec_aug @ Wh_aug streamed in chunks ----
            ps2 = tc.tile_pool(name="ps2", bufs=3, space="PSUM")
            p2 = ps2.__enter__()
            w_gb = w_all[:].rearrange("i b o t -> i (o t) b")  # [96, 864, 8]
            for jd in range(NCH):
                whc = wh_p.tile([E + 1, CHW], F16, tag="whc")
                nc.sync.dma_start(whc[:], wh_d[:, jd * CHW:(jd + 1) * CHW])
                pwg = p2.tile([DIM, GRP * BL], F32, name="pwg", tag="pwg")
                for gg in range(GRP):
                    nc.tensor.matmul(pwg[:, gg * BL:(gg + 1) * BL],
                                     whc[:, gg * DIM:(gg + 1) * DIM], pvec[:],
                                     start=True, stop=True)
                g0 = jd * GRP
                if jd % 2 == 0:
                    nc.vector.tensor_copy(w_gb[:, g0:g0 + GRP, :],
                                          pwg[:].rearrange("i (g b) -> i g b", g=GRP))
                else:
                    nc.scalar.activation(w_gb[:, g0:g0 + GRP, :],
                                         pwg[:].rearrange("i (g b) -> i g b", g=GRP),
                                         AF.Copy)
            ps2.__exit__(None, None, None)

            # ---- phase 4+5: per-sample conv, gelu, output projection ----
            ps3 = tc.tile_pool(name="ps3", bufs=4, space="PSUM")
            p3 = ps3.__enter__()
            rot = 0
            for b in range(BL):
                for h2 in range(2):
                    py = p3.tile([DIM, HF], F32, name="py", tag="py")
                    for t in range(9):
                        dy, dx = t // 3, t % 3
                        nc.tensor.matmul(
                            py[:], w_all[:, b, :, t],
                            xd_pad[:, b, h2 * 14 + dy:h2 * 14 + dy + 14, dx:dx + 28],
                            start=(t == 0), stop=(t == 8))
                    nc.scalar.activation(y_all[:, b, h2 * HF:(h2 + 1) * HF], py[:],
                                         AF.Gelu_apprx_sigmoid)
                ob = ob_p.tile([128, 3, 2, HF], F16, tag="ob")
                for h2 in range(2):
                    for c in range(3):
                        po = p3.tile([128, HF], F32, name="po", tag="po")
                        nc.tensor.matmul(po[:], wu_sb[:, c * 128:(c + 1) * 128],
                                         y_all[:, b, h2 * HF:(h2 + 1) * HF],
                                         start=True, stop=True)
                        dst = ob[:, c, h2, :]
                        bcol = bias_sb[:, 2 + c:3 + c]
                        if rot % 3 == 0:
                            nc.scalar.activation(dst, po[:], AF.Identity, bias=bcol)
                        else:
                            nc.vector.tensor_scalar_add(dst, po[:], bcol)
                        rot += 1
                nc.sync.dma_start(yo_d[:, :, b, :],
                                  ob[:].rearrange("p c h q -> p c (h q)"))
            ps3.__exit__(None, None, None)

    nc.compile()
    return nc


_NC_CACHE = None


def _get_nc():
    global _NC_CACHE
    if _NC_CACHE is None:
        _NC_CACHE = build_nc()
    return _NC_CACHE


def _prep_inputs(x, Wd, bd, Wm1, bm1, Wm2, bm2, Wh, bh, emb, Wu, bu):
    """Host-side prep: pure layout/dtype transforms + x-independent weight
    folding (w0 row, bias packing)."""
    f16 = np.float16
    Wh = np.asarray(Wh, np.float32)
    w0 = (np.asarray(emb, np.float32) + np.asarray(bm2, np.float32)) @ Wh \
        + np.asarray(bh, np.float32)
    whp = Wh.reshape(E, DIM, DIM, KK * KK).transpose(0, 1, 3, 2).reshape(E, WH_COLS)
    w0p = w0.reshape(DIM, DIM, KK * KK).transpose(0, 2, 1).reshape(1, WH_COLS)
    wh_aug = np.concatenate([whp, w0p], 0).astype(f16)

    w16 = np.zeros((128, 552), f16)
    w16[:, 0:192] = np.asarray(Wm1, np.float32).reshape(3, 128, E) \
        .transpose(1, 0, 2).reshape(128, 192)
    w16[:, 192:480] = np.asarray(Wd, np.float32).reshape(3, 128, DIM) \
        .transpose(1, 0, 2).reshape(128, 288)
    w16[0:E, 480:544] = np.asarray(Wm2, np.float32)

    bias = np.zeros((128, 5), np.float32)
    bias[0:E, 0] = np.asarray(bm1, np.float32)
    bias[0:DIM, 1] = np.asarray(bd, np.float32)
    bias[:, 2:5] = np.asarray(bu, np.float32).reshape(3, 128).T

    shared = {"w16": w16, "wu": np.asarray(Wu, np.float32).astype(f16),
              "bias": bias, "wh": wh_aug}
    xs = np.asarray(x, np.float32).astype(f16).reshape(B, P, C)
    in_maps = []
    for k in range(NCORES):
        xt = xs[k * BL:(k + 1) * BL].reshape(BL, P, 3, 128).transpose(3, 2, 0, 1)
        m = dict(shared)
        m["xt"] = np.ascontiguousarray(xt)
        in_maps.append(m)
    return in_maps


def _run(inputs, **spmd_kwargs):
    nc = _get_nc()
    in_maps = _prep_inputs(**inputs)
    res = run_bass_kernel_spmd(nc, in_maps, core_ids=list(range(NCORES)), **spmd_kwargs)
    parts = []
    for r in res.results:
        yo = np.asarray(r["yo"])                      # [128, 3, BL, P] fp16
        parts.append(yo.transpose(2, 3, 1, 0).reshape(BL, P, C))
    out = np.concatenate(parts, 0).astype(np.float32)
    return out.reshape(B, H, W, C), res


def kernel(**inputs) -> np.ndarray:
    out, _ = _run(inputs)
    return out


# revision 22
# speedup vs baseline: 1.3835x; 1.1130x over previous
"""Trainium2 Bass kernel for Convpass-swin hypernet fused adapter.

Reference computation (per batch sample):
  h      = relu(x @ Wm1 + bm1)                    # [B,H,W,64]
  prompt = mean_hw(h) @ Wm2 + bm2                 # [B,64]  (mean commutes with matmul)
  wflat  = (emb + prompt) @ Wh + bh               # [B,96*96*9]
  xd     = quickgelu(x @ Wd + bd)                 # [B,H,W,96]
  y      = quickgelu(conv3x3(xd, wflat))          # per-sample dynamic grouped conv
  out    = y @ Wu + bu                            # [B,H,W,384]

Sharding: data-parallel over batch B=64 across 8 cores (8 samples/core),
weights replicated.

Key structure (all host-side prep is x-independent weight folding plus pure
dtype/layout casts of x):
  * x is uploaded pre-transposed as [C_part=128, 3, BL, P] fp16, so no
    on-chip transposes are needed; meta/adapter matmuls read it directly.
  * The hypernet constant term w0 = (emb+bm2)@Wh + bh is folded into an
    extra row of Wh; the device computes wflat = prompt_raw @ Wh + w0 via an
    augmented [65,BL] pvec in fp16.
  * Wh is permuted TAP-MAJOR (t,o,i): while the 10.8 MB matrix streams from
    HBM, six (sample,half) conv accumulations live in persistent PSUM banks
    and consume each tap as soon as its weights land, hiding a third of the
    conv behind the DMA stream. Remaining samples run after the stream.
  * The output projection runs with C on partitions (stationary Wu^T
    chunks), so bu is a per-partition bias on the PSUM->SBUF copy and the
    output goes back as [C_part, 3, BL, P] fp16 (host transposes back).
"""
import numpy as np

import concourse.bass as bass
import concourse.tile as tile
import concourse.mybir as mybir
from concourse import bacc
from concourse.bass_utils import run_bass_kernel_spmd

F32 = mybir.dt.float32
F16 = mybir.dt.float16
AF = mybir.ActivationFunctionType
AX = mybir.AxisListType

# problem constants
B, H, W, C = 64, 28, 28, 384
DIM, E, KK = 96, 64, 3
NCORES = 8
BL = B // NCORES          # samples per core
P = H * W                 # 784 positions per sample
HP = H + 2                # padded spatial
WH_COLS = DIM * DIM * 9   # 82944
NCH = 27                  # Wh stream chunks
CHW = WH_COLS // NCH      # 3072 columns per chunk
GRP = CHW // DIM          # 32 (t,o) groups per chunk
HF = P // 2               # 392 positions per half
NRES = 3                  # samples whose conv rides the Wh stream (2 PSUM banks each)


def build_nc():
    nc = bacc.Bacc("TRN2", target_bir_lowering=False, debug=False)

    xt_d = nc.dram_tensor("xt", [128, 3, BL, P], F16, kind="ExternalInput").ap()
    w16_d = nc.dram_tensor("w16", [128, 936], F16, kind="ExternalInput").ap()
    bias_d = nc.dram_tensor("bias", [128, 5], F32, kind="ExternalInput").ap()
    wh_d = nc.dram_tensor("wh", [E + 1, WH_COLS], F16, kind="ExternalInput").ap()
    yo_d = nc.dram_tensor("yo", [128, 3, BL, P], F16, kind="ExternalOutput").ap()

    with tile.TileContext(nc) as tc:
        with (
            tc.tile_pool(name="const", bufs=1) as cp,
            tc.tile_pool(name="persist", bufs=1) as pp,
            tc.tile_pool(name="scr", bufs=3) as sp,
            tc.tile_pool(name="wh", bufs=16) as wh_p,
            tc.tile_pool(name="ob", bufs=3) as ob_p,
        ):
            # ---- constants (two DMAs), then the x stream ----
            w16_sb = cp.tile([128, 936], F16)
            nc.sync.dma_start(w16_sb[:], w16_d[:])
            bias_sb = cp.tile([128, 5], F32)
            nc.sync.dma_start(bias_sb[:], bias_d[:])

            xt_sb = pp.tile([128, 3, BL, P], F16)
            for b in range(BL):
                nc.sync.dma_start(xt_sb[:, :, b, :], xt_d[:, :, b, :])

            # ---- persistent state ----
            xd_pad = pp.tile([DIM, BL, HP, HP], F16)      # padded gelu(x@Wd+bd)
            w_all = pp.tile([DIM, BL, 9, DIM], F16)       # per-sample conv weights [i,b,t,o]
            y_all = pp.tile([DIM, BL, P], F16)            # gelu(conv) activations
            hsum = pp.tile([E, BL, 2], F32)               # per-(sample,half) relu sums
            hbar = pp.tile([E, BL], F32)
            hbar16 = pp.tile([E, BL], F16)
            pvec = pp.tile([E + 1, BL], F16)              # (prompt_raw, 1)^T

            # zero only the conv halo ring; phase 1b writes the interior
            nc.vector.memset(xd_pad[:, :, 0:1, :], 0.0)
            nc.vector.memset(xd_pad[:, :, HP - 1:HP, :], 0.0)
            nc.vector.memset(xd_pad[:, :, 1:HP - 1, 0:1], 0.0)
            nc.vector.memset(xd_pad[:, :, 1:HP - 1, HP - 1:HP], 0.0)
            nc.vector.memset(pvec[E:E + 1, :], 1.0)

            # pwg pool opens first so pool releases stay LIFO
            ps2 = tc.tile_pool(name="ps2", bufs=1, space="PSUM")
            p2 = ps2.__enter__()
            w_gb = w_all[:].rearrange("i b t o -> i (t o) b")  # [96, 864, 8], tap-major

            # ---- phase 1a: meta h = relu(x@Wm1+bm1), spatial sums ----
            ps1 = tc.tile_pool(name="ps1", bufs=1, space="PSUM")
            p1 = ps1.__enter__()
            for b in range(BL):
                for h2 in range(2):
                    ph = p1.tile([E, HF], F32, name="ph", tag="ph", bufs=2)
                    for c in range(3):
                        nc.tensor.matmul(ph[:], w16_sb[:, c * 64:(c + 1) * 64],
                                         xt_sb[:, c, b, h2 * HF:(h2 + 1) * HF],
                                         start=(c == 0), stop=(c == 2))
                    h_scr = sp.tile([E, HF], F32, tag="hscr")
                    nc.scalar.activation(h_scr[:], ph[:], AF.Relu,
                                         bias=bias_sb[0:E, 0:1],
                                         accum_out=hsum[:, b, h2:h2 + 1])
            nc.vector.reduce_sum(hbar[:], hsum[:], axis=AX.X)
            nc.scalar.activation(hbar16[:], hbar[:], AF.Copy, scale=1.0 / P)

            # ---- phase 1b + prompt + Wh stream start, interleaved ----
            def emit_xd(b):
                for h2 in range(2):
                    px = p1.tile([DIM, HF], F32, name="px", tag="px", bufs=2)
                    for c in range(3):
                        nc.tensor.matmul(px[:], w16_sb[:, 192 + c * 96:192 + (c + 1) * 96],
                                         xt_sb[:, c, b, h2 * HF:(h2 + 1) * HF],
                                         start=(c == 0), stop=(c == 2))
                    nc.scalar.activation(
                        xd_pad[:, b, 1 + h2 * 14:15 + h2 * 14, 1:29],
                        px[:].rearrange("p (r c) -> p r c", r=14),
                        AF.Gelu_apprx_sigmoid, bias=bias_sb[0:DIM, 1:2])

            emit_xd(0)
            ppm = p1.tile([E, BL], F32, name="ppm", tag="ppm", bufs=1)
            nc.tensor.matmul(ppm[:], w16_sb[0:E, 480:544], hbar16[:],
                             start=True, stop=True)
            nc.scalar.activation(pvec[0:E, :], ppm[:], AF.Copy)

            def emit_ph3(jd):
                whc = wh_p.tile([E + 1, CHW], F16, tag="whc")
                nc.sync.dma_start(whc[:], wh_d[:, jd * CHW:(jd + 1) * CHW])
                pwg = p2.tile([DIM, GRP * BL], F32, name="pwg", tag="pwg", bufs=3)
                for gg in range(GRP):
                    nc.tensor.matmul(pwg[:, gg * BL:(gg + 1) * BL],
                                     whc[:, gg * DIM:(gg + 1) * DIM], pvec[:],
                                     start=True, stop=True)
                g0 = jd * GRP
                src = pwg[:].rearrange("i (g b) -> i g b", g=GRP)
                if jd % 2 == 0:
                    nc.vector.tensor_copy(w_gb[:, g0:g0 + GRP, :], src)
                else:
                    nc.scalar.activation(w_gb[:, g0:g0 + GRP, :], src, AF.Copy)

            jd = 0
            for b in range(1, BL):
                emit_xd(b)
                emit_ph3(jd); emit_ph3(jd + 1)
                jd += 2
            ps1.__exit__(None, None, None)

            # ---- resident conv: taps accumulate while Wh streams ----
            res = tc.tile_pool(name="res", bufs=1, space="PSUM")
            pr = res.__enter__()
            RES_PAIRS = [(0, 0), (0, 1), (1, 0), (1, 1), (2, 0)]
            py_res = [pr.tile([128, HF], F32, name=f"pyr{i}")
                      for i in range(len(RES_PAIRS))]

            def emit_res_tap(t):
                for i, (b, h2) in enumerate(RES_PAIRS):
                    dy, dx = t // 3, t % 3
                    nc.tensor.matmul(
                        py_res[i][0:DIM, :], w_all[:, b, t, :],
                        xd_pad[:, b, h2 * 14 + dy:h2 * 14 + dy + 14, dx:dx + 28],
                        start=(t == 0), stop=(t == 8))

            tap_next = 0
            # copies are emitted through chunk jd when jd is odd (pair flush)
            # or at the final chunk; taps may only consume copied weights
            def emit_ready_taps(jc):
                nonlocal tap_next
                while tap_next < 9 and 3 * tap_next + 2 <= jc:
                    emit_res_tap(tap_next)
                    tap_next += 1

            emit_ready_taps(jd - 5)
            while jd < NCH:
                emit_ph3(jd)
                if jd % 2 == 1 or jd == NCH - 1:
                    # two-pair emission lag so the PE never stalls on a copy
                    emit_ready_taps(jd - 4)
                jd += 1
            emit_ready_taps(NCH - 1)
            for i, (b, h2) in enumerate(RES_PAIRS):
                nc.scalar.activation(y_all[:, b, h2 * HF:(h2 + 1) * HF],
                                     py_res[i][0:DIM, :], AF.Gelu_apprx_sigmoid)

            # ---- tail: remaining convs + output projection for all samples ----
            # non-resident convs rotate through the resident PSUM tiles (the
            # write only waits for that tile's gelu read, not the whole pool)
            rot = 0
            pyr_rot = [0]

            def emit_out(b, ob):
                nonlocal rot
                for h2 in range(2):
                    for c in range(3):
                        po = py_res[2 + po_rot[0] % 3]
                        po_rot[0] += 1
                        nc.tensor.matmul(po[:], w16_sb[0:DIM, 552 + c * 128:552 + (c + 1) * 128],
                                         y_all[:, b, h2 * HF:(h2 + 1) * HF],
                                         start=True, stop=True)
                        dst = ob[:, c, h2, :]
                        bcol = bias_sb[:, 2 + c:3 + c]
                        if rot % 2 == 0:
                            nc.scalar.activation(dst, po[:], AF.Identity, bias=bcol)
                        else:
                            nc.vector.tensor_scalar_add(dst, po[:], bcol)
                        rot += 1
                    nc.sync.dma_start(yo_d[:, :, b, h2 * HF:(h2 + 1) * HF],
                                      ob[:, :, h2, :])

            po_rot = [0]

            def emit_conv_half(b, h2):
                py = py_res[pyr_rot[0] % 2]
                pyr_rot[0] += 1
                for t in range(9):
                    dy, dx = t // 3, t % 3
                    nc.tensor.matmul(
                        py[0:DIM, :], w_all[:, b, t, :],
                        xd_pad[:, b, h2 * 14 + dy:h2 * 14 + dy + 14, dx:dx + 28],
                        start=(t == 0), stop=(t == 8))
                nc.scalar.activation(y_all[:, b, h2 * HF:(h2 + 1) * HF],
                                     py[0:DIM, :], AF.Gelu_apprx_sigmoid)

            def emit_conv(b):
                for h2 in range(2):
                    py = py_res[pyr_rot[0] % 2]
                    pyr_rot[0] += 1
                    for t in range(9):
                        dy, dx = t // 3, t % 3
                        nc.tensor.matmul(
                            py[0:DIM, :], w_all[:, b, t, :],
                            xd_pad[:, b, h2 * 14 + dy:h2 * 14 + dy + 14, dx:dx + 28],
                            start=(t == 0), stop=(t == 8))
                    nc.scalar.activation(y_all[:, b, h2 * HF:(h2 + 1) * HF],
                                         py[0:DIM, :], AF.Gelu_apprx_sigmoid)

            # software pipeline: PE stays on convs while gelu/copy engines
            # drain the previous sample's projection
            out_q = 0
            emit_conv_half(2, 1)
            for b in range(NRES, BL):
                emit_conv(b)
                ob = ob_p.tile([128, 3, 2, HF], F16, tag="ob")
                emit_out(out_q, ob)
                out_q += 1
            while out_q < BL:
                ob = ob_p.tile([128, 3, 2, HF], F16, tag="ob")
                emit_out(out_q, ob)
                out_q += 1
            res.__exit__(None, None, None)
            ps2.__exit__(None, None, None)

    nc.compile()
    return nc


_NC_CACHE = None


def _get_nc():
    global _NC_CACHE
    if _NC_CACHE is None:
        _NC_CACHE = build_nc()
    return _NC_CACHE


def _prep_inputs(x, Wd, bd, Wm1, bm1, Wm2, bm2, Wh, bh, emb, Wu, bu):
    """Host-side prep: pure layout/dtype transforms + x-independent weight
    folding (w0 row, bias packing)."""
    f16 = np.float16
    Wh = np.asarray(Wh, np.float32)
    w0 = (np.asarray(emb, np.float32) + np.asarray(bm2, np.float32)) @ Wh \
        + np.asarray(bh, np.float32)
    # (e, o, i, t) -> tap-major (e, t, o, i)
    whp = Wh.reshape(E, DIM, DIM, KK * KK).transpose(0, 3, 1, 2).reshape(E, WH_COLS)
    w0p = w0.reshape(DIM, DIM, KK * KK).transpose(2, 0, 1).reshape(1, WH_COLS)
    wh_aug = np.concatenate([whp, w0p], 0).astype(f16)

    w16 = np.zeros((128, 936), f16)
    w16[:, 0:192] = np.asarray(Wm1, np.float32).reshape(3, 128, E) \
        .transpose(1, 0, 2).reshape(128, 192)
    w16[:, 192:480] = np.asarray(Wd, np.float32).reshape(3, 128, DIM) \
        .transpose(1, 0, 2).reshape(128, 288)
    w16[0:E, 480:544] = np.asarray(Wm2, np.float32)
    w16[0:DIM, 552:936] = np.asarray(Wu, np.float32)

    bias = np.zeros((128, 5), np.float32)
    bias[0:E, 0] = np.asarray(bm1, np.float32)
    bias[0:DIM, 1] = np.asarray(bd, np.float32)
    bias[:, 2:5] = np.asarray(bu, np.float32).reshape(3, 128).T

    shared = {"w16": w16, "bias": bias, "wh": wh_aug}
    xs = np.asarray(x, np.float32).astype(f16).reshape(B, P, C)
    in_maps = []
    for k in range(NCORES):
        xt = xs[k * BL:(k + 1) * BL].reshape(BL, P, 3, 128).transpose(3, 2, 0, 1)
        m = dict(shared)
        m["xt"] = np.ascontiguousarray(xt)
        in_maps.append(m)
    return in_maps


def _run(inputs, **spmd_kwargs):
    nc = _get_nc()
    in_maps = _prep_inputs(**inputs)
    res = run_bass_kernel_spmd(nc, in_maps, core_ids=list(range(NCORES)), **spmd_kwargs)
    parts = []
    for r in res.results:
        yo = np.asarray(r["yo"])                      # [128, 3, BL, P] fp16
        parts.append(yo.transpose(2, 3, 1, 0).reshape(BL, P, C))
    out = np.concatenate(parts, 0).astype(np.float32)
    return out.reshape(B, H, W, C), res


def kernel(**inputs) -> np.ndarray:
    out, _ = _run(inputs)
    return out


# revision 28
# speedup vs baseline: 1.4029x; 1.0140x over previous
"""Trainium2 Bass kernel for Convpass-swin hypernet fused adapter.

Reference computation (per batch sample):
  h      = relu(x @ Wm1 + bm1)                    # [B,H,W,64]
  prompt = mean_hw(h) @ Wm2 + bm2                 # [B,64]  (mean commutes with matmul)
  wflat  = (emb + prompt) @ Wh + bh               # [B,96*96*9]
  xd     = quickgelu(x @ Wd + bd)                 # [B,H,W,96]
  y      = quickgelu(conv3x3(xd, wflat))          # per-sample dynamic grouped conv
  out    = y @ Wu + bu                            # [B,H,W,384]

Sharding: data-parallel over batch B=64 across 8 cores (8 samples/core),
weights replicated.

Key structure (all host-side prep is x-independent weight folding plus pure
dtype/layout casts of x):
  * x is uploaded pre-transposed as [C_part=128, 3, BL, P] fp16, so no
    on-chip transposes are needed; meta/adapter matmuls read it directly.
  * The hypernet constant term w0 = (emb+bm2)@Wh + bh is folded into an
    extra row of Wh; the device computes wflat = prompt_raw @ Wh + w0 via an
    augmented [65,BL] pvec in fp16.
  * Wh is permuted TAP-MAJOR (t,o,i): while the 10.8 MB matrix streams from
    HBM, six (sample,half) conv accumulations live in persistent PSUM banks
    and consume each tap as soon as its weights land, hiding a third of the
    conv behind the DMA stream. Remaining samples run after the stream.
  * The output projection runs with C on partitions (stationary Wu^T
    chunks), so bu is a per-partition bias on the PSUM->SBUF copy and the
    output goes back as [C_part, 3, BL, P] fp16 (host transposes back).
"""
import numpy as np

import concourse.bass as bass
import concourse.tile as tile
import concourse.mybir as mybir
from concourse import bacc
from concourse.bass_utils import run_bass_kernel_spmd

F32 = mybir.dt.float32
F16 = mybir.dt.float16
AF = mybir.ActivationFunctionType
AX = mybir.AxisListType

# problem constants
B, H, W, C = 64, 28, 28, 384
DIM, E, KK = 96, 64, 3
NCORES = 8
BL = B // NCORES          # samples per core
P = H * W                 # 784 positions per sample
HP = H + 2                # padded spatial
WH_COLS = DIM * DIM * 9   # 82944
NCH = 27                  # Wh stream chunks
CHW = WH_COLS // NCH      # 3072 columns per chunk
GRP = CHW // DIM          # 32 (t,o) groups per chunk
HF = P // 2               # 392 positions per half
NRES = 3                  # samples whose conv rides the Wh stream (2 PSUM banks each)


def build_nc():
    nc = bacc.Bacc("TRN2", target_bir_lowering=False, debug=False)

    xt_d = nc.dram_tensor("xt", [128, 3, BL, P], F16, kind="ExternalInput").ap()
    w16_d = nc.dram_tensor("w16", [128, 936], F16, kind="ExternalInput").ap()
    bias_d = nc.dram_tensor("bias", [128, 5], F32, kind="ExternalInput").ap()
    wh_d = nc.dram_tensor("wh", [E + 1, WH_COLS], F16, kind="ExternalInput").ap()
    yo_d = nc.dram_tensor("yo", [128, 3, BL, P], F16, kind="ExternalOutput").ap()

    with tile.TileContext(nc) as tc:
        with (
            tc.tile_pool(name="const", bufs=1) as cp,
            tc.tile_pool(name="persist", bufs=1) as pp,
            tc.tile_pool(name="scr", bufs=3) as sp,
            tc.tile_pool(name="wh", bufs=16) as wh_p,
            tc.tile_pool(name="ob", bufs=4) as ob_p,
        ):
            # ---- constants (two DMAs), then the x stream ----
            w16_sb = cp.tile([128, 936], F16)
            nc.sync.dma_start(w16_sb[:], w16_d[:])
            bias_sb = cp.tile([128, 5], F32)
            nc.sync.dma_start(bias_sb[:], bias_d[:])

            xt_sb = pp.tile([128, 3, BL, P], F16)
            for b in range(BL):
                nc.sync.dma_start(xt_sb[:, :, b, :], xt_d[:, :, b, :])

            # ---- persistent state ----
            xd_pad = pp.tile([DIM, BL, HP, HP], F16)      # padded gelu(x@Wd+bd)
            w_all = pp.tile([DIM, BL, 9, DIM], F16)       # per-sample conv weights [i,b,t,o]
            y_all = pp.tile([DIM, BL, P], F16)            # gelu(conv) activations
            hsum = pp.tile([E, BL, 2], F32)               # per-(sample,half) relu sums
            hbar = pp.tile([E, BL], F32)
            hbar16 = pp.tile([E, BL], F16)
            pvec = pp.tile([E + 1, BL], F16)              # (prompt_raw, 1)^T

            # zero only the conv halo ring; phase 1b writes the interior
            nc.vector.memset(xd_pad[:, :, 0:1, :], 0.0)
            nc.vector.memset(xd_pad[:, :, HP - 1:HP, :], 0.0)
            nc.vector.memset(xd_pad[:, :, 1:HP - 1, 0:1], 0.0)
            nc.vector.memset(xd_pad[:, :, 1:HP - 1, HP - 1:HP], 0.0)
            nc.vector.memset(pvec[E:E + 1, :], 1.0)

            # pwg pool opens first so pool releases stay LIFO
            ps2 = tc.tile_pool(name="ps2", bufs=1, space="PSUM")
            p2 = ps2.__enter__()
            w_gb = w_all[:].rearrange("i b t o -> i (t o) b")  # [96, 864, 8], tap-major

            # ---- phase 1a: meta h = relu(x@Wm1+bm1), spatial sums ----
            ps1 = tc.tile_pool(name="ps1", bufs=1, space="PSUM")
            p1 = ps1.__enter__()
            for b in range(BL):
                for h2 in range(2):
                    ph = p1.tile([E, HF], F32, name="ph", tag="ph", bufs=2)
                    for c in range(3):
                        nc.tensor.matmul(ph[:], w16_sb[:, c * 64:(c + 1) * 64],
                                         xt_sb[:, c, b, h2 * HF:(h2 + 1) * HF],
                                         start=(c == 0), stop=(c == 2))
                    h_scr = sp.tile([E, HF], F32, tag="hscr")
                    nc.scalar.activation(h_scr[:], ph[:], AF.Relu,
                                         bias=bias_sb[0:E, 0:1],
                                         accum_out=hsum[:, b, h2:h2 + 1])
            nc.vector.reduce_sum(hbar[:], hsum[:], axis=AX.X)
            nc.scalar.activation(hbar16[:], hbar[:], AF.Copy, scale=1.0 / P)

            # ---- phase 1b + prompt + Wh stream start, interleaved ----
            def emit_xd(b):
                for h2 in range(2):
                    px = p1.tile([DIM, HF], F32, name="px", tag="px", bufs=2)
                    for c in range(3):
                        nc.tensor.matmul(px[:], w16_sb[:, 192 + c * 96:192 + (c + 1) * 96],
                                         xt_sb[:, c, b, h2 * HF:(h2 + 1) * HF],
                                         start=(c == 0), stop=(c == 2))
                    nc.scalar.activation(
                        xd_pad[:, b, 1 + h2 * 14:15 + h2 * 14, 1:29],
                        px[:].rearrange("p (r c) -> p r c", r=14),
                        AF.Gelu_apprx_sigmoid, bias=bias_sb[0:DIM, 1:2])

            emit_xd(0)
            ppm = p1.tile([E, BL], F32, name="ppm", tag="ppm", bufs=1)
            nc.tensor.matmul(ppm[:], w16_sb[0:E, 480:544], hbar16[:],
                             start=True, stop=True)
            nc.scalar.activation(pvec[0:E, :], ppm[:], AF.Copy)

            def emit_ph3(jd):
                whc = wh_p.tile([E + 1, CHW], F16, tag="whc")
                nc.sync.dma_start(whc[:], wh_d[:, jd * CHW:(jd + 1) * CHW])
                pwg = p2.tile([DIM, GRP * BL], F32, name="pwg", tag="pwg", bufs=3)
                for gg in range(GRP):
                    nc.tensor.matmul(pwg[:, gg * BL:(gg + 1) * BL],
                                     whc[:, gg * DIM:(gg + 1) * DIM], pvec[:],
                                     start=True, stop=True)
                g0 = jd * GRP
                src = pwg[:].rearrange("i (g b) -> i g b", g=GRP)
                if jd % 2 == 0:
                    nc.vector.tensor_copy(w_gb[:, g0:g0 + GRP, :], src)
                else:
                    nc.scalar.activation(w_gb[:, g0:g0 + GRP, :], src, AF.Copy)

            jd = 0
            for b in range(1, BL):
                emit_xd(b)
                emit_ph3(jd); emit_ph3(jd + 1)
                jd += 2
            ps1.__exit__(None, None, None)

            # ---- resident conv: taps accumulate while Wh streams ----
            res = tc.tile_pool(name="res", bufs=1, space="PSUM")
            pr = res.__enter__()
            RES_PAIRS = [(0, 0), (0, 1), (1, 0), (1, 1), (2, 0)]
            py_res = [pr.tile([128, HF], F32, name=f"pyr{i}")
                      for i in range(len(RES_PAIRS))]

            def emit_res_tap(t):
                for i, (b, h2) in enumerate(RES_PAIRS):
                    dy, dx = t // 3, t % 3
                    nc.tensor.matmul(
                        py_res[i][0:DIM, :], w_all[:, b, t, :],
                        xd_pad[:, b, h2 * 14 + dy:h2 * 14 + dy + 14, dx:dx + 28],
                        start=(t == 0), stop=(t == 8))

            tap_next = 0
            # copies are emitted through chunk jd when jd is odd (pair flush)
            # or at the final chunk; taps may only consume copied weights
            def emit_ready_taps(jc):
                nonlocal tap_next
                while tap_next < 9 and 3 * tap_next + 2 <= jc:
                    emit_res_tap(tap_next)
                    tap_next += 1

            emit_ready_taps(jd - 5)
            while jd < NCH:
                emit_ph3(jd)
                if jd % 2 == 1 or jd == NCH - 1:
                    # two-pair emission lag so the PE never stalls on a copy
                    emit_ready_taps(jd - 4)
                jd += 1
            emit_ready_taps(NCH - 1)
            for i, (b, h2) in enumerate(RES_PAIRS):
                nc.scalar.activation(y_all[:, b, h2 * HF:(h2 + 1) * HF],
                                     py_res[i][0:DIM, :], AF.Gelu_apprx_sigmoid)

            # ---- tail: remaining convs + output projection for all samples ----
            # non-resident convs rotate through the resident PSUM tiles (the
            # write only waits for that tile's gelu read, not the whole pool)
            rot = 0
            pyr_rot = [0]

            def emit_out_half(b, ob, h2):
                nonlocal rot
                for c in range(3):
                    po = py_res[2 + po_rot[0] % 3]
                    po_rot[0] += 1
                    nc.tensor.matmul(po[:], w16_sb[0:DIM, 552 + c * 128:552 + (c + 1) * 128],
                                     y_all[:, b, h2 * HF:(h2 + 1) * HF],
                                     start=True, stop=True)
                    dst = ob[:, c, h2, :]
                    bcol = bias_sb[:, 2 + c:3 + c]
                    if rot % 2 == 0:
                        nc.scalar.activation(dst, po[:], AF.Identity, bias=bcol)
                    else:
                        nc.vector.tensor_scalar_add(dst, po[:], bcol)
                    rot += 1
                nc.sync.dma_start(yo_d[:, :, b, h2 * HF:(h2 + 1) * HF],
                                  ob[:, :, h2, :])

            po_rot = [0]

            def emit_conv_half(b, h2):
                py = py_res[pyr_rot[0] % 2]
                pyr_rot[0] += 1
                for t in range(9):
                    dy, dx = t // 3, t % 3
                    nc.tensor.matmul(
                        py[0:DIM, :], w_all[:, b, t, :],
                        xd_pad[:, b, h2 * 14 + dy:h2 * 14 + dy + 14, dx:dx + 28],
                        start=(t == 0), stop=(t == 8))
                nc.scalar.activation(y_all[:, b, h2 * HF:(h2 + 1) * HF],
                                     py[0:DIM, :], AF.Gelu_apprx_sigmoid)

            # software pipeline: out-halves interleave between conv-halves so
            # the projection's psum ring and copies never gate the PE
            conv_halves = [(2, 1)] + [(b, h2) for b in range(NRES, BL)
                                      for h2 in range(2)]
            out_halves = [(b, h2) for b in range(BL) for h2 in range(2)]
            oq = 0
            obs = {}

            def emit_next_out():
                nonlocal oq
                b, h2 = out_halves[oq]
                if h2 == 0:
                    obs[b] = ob_p.tile([128, 3, 2, HF], F16, name=f"ob{b}", tag="ob")
                emit_out_half(b, obs[b], h2)
                oq += 1

            for b, h2 in conv_halves:
                emit_conv_half(b, h2)
                if oq < len(out_halves):
                    emit_next_out()
            while oq < len(out_halves):
                emit_next_out()
            res.__exit__(None, None, None)
            ps2.__exit__(None, None, None)

    nc.compile()
    return nc


_NC_CACHE = None


def _get_nc():
    global _NC_CACHE
    if _NC_CACHE is None:
        _NC_CACHE = build_nc()
    return _NC_CACHE


def _prep_inputs(x, Wd, bd, Wm1, bm1, Wm2, bm2, Wh, bh, emb, Wu, bu):
    """Host-side prep: pure layout/dtype transforms + x-independent weight
    folding (w0 row, bias packing)."""
    f16 = np.float16
    Wh = np.asarray(Wh, np.float32)
    w0 = (np.asarray(emb, np.float32) + np.asarray(bm2, np.float32)) @ Wh \
        + np.asarray(bh, np.float32)
    # (e, o, i, t) -> tap-major (e, t, o, i)
    whp = Wh.reshape(E, DIM, DIM, KK * KK).transpose(0, 3, 1, 2).reshape(E, WH_COLS)
    w0p = w0.reshape(DIM, DIM, KK * KK).transpose(2, 0, 1).reshape(1, WH_COLS)
    wh_aug = np.concatenate([whp, w0p], 0).astype(f16)

    w16 = np.zeros((128, 936), f16)
    w16[:, 0:192] = np.asarray(Wm1, np.float32).reshape(3, 128, E) \
        .transpose(1, 0, 2).reshape(128, 192)
    w16[:, 192:480] = np.asarray(Wd, np.float32).reshape(3, 128, DIM) \
        .transpose(1, 0, 2).reshape(128, 288)
    w16[0:E, 480:544] = np.asarray(Wm2, np.float32)
    w16[0:DIM, 552:936] = np.asarray(Wu, np.float32)

    bias = np.zeros((128, 5), np.float32)
    bias[0:E, 0] = np.asarray(bm1, np.float32)
    bias[0:DIM, 1] = np.asarray(bd, np.float32)
    bias[:, 2:5] = np.asarray(bu, np.float32).reshape(3, 128).T

    shared = {"w16": w16, "bias": bias, "wh": wh_aug}
    xs = np.asarray(x, np.float32).astype(f16).reshape(B, P, C)
    in_maps = []
    for k in range(NCORES):
        xt = xs[k * BL:(k + 1) * BL].reshape(BL, P, 3, 128).transpose(3, 2, 0, 1)
        m = dict(shared)
        m["xt"] = np.ascontiguousarray(xt)
        in_maps.append(m)
    return in_maps


def _run(inputs, **spmd_kwargs):
    nc = _get_nc()
    in_maps = _prep_inputs(**inputs)
    res = run_bass_kernel_spmd(nc, in_maps, core_ids=list(range(NCORES)), **spmd_kwargs)
    parts = []
    for r in res.results:
        yo = np.asarray(r["yo"])                      # [128, 3, BL, P] fp16
        parts.append(yo.transpose(2, 3, 1, 0).reshape(BL, P, C))
    out = np.concatenate(parts, 0).astype(np.float32)
    return out.reshape(B, H, W, C), res


def kernel(**inputs) -> np.ndarray:
    out, _ = _run(inputs)
    return out
